# revision 1
# baseline (speedup 1.0000x reference)
"""MoE FFN (D=1024, F=4096, E=4, top-2) Trainium2 Bass kernel.

Strategy: data-parallel over tokens across 8 NeuronCores (1024 tokens/core,
expert weights replicated). Per core, everything is computed in the
"tokens-on-free-dim" orientation so only x needs a transpose:

  xT[D, T]   = PE-transpose(x)                (fp32 exact + fp32r copy)
  logits[E,T]= Wr^T @ xT                      (fp32 matmuls - exact top-2)
  top-2 mask, renormalized gates g[E, T]      (DVE/ACT ops in token space)
  G[e]       = bcast(g[e]) over partitions    (stride-0 DMA via DRAM)
  h[F, T]    = gelu(W1[e]^T @ xT + b1[e])     (fp32r matmuls, ACT gelu)
  o[D, T]    = W2[e]^T @ h                    (fp32r matmuls)
  acc        = b2^T @ g + sum_e G[e] * o[e]   (DVE combine)
  out        = PE-transpose(acc)

fp32r (TF32) runs the 128x128 PE at 1 cycle/row vs 4 for fp32.
"""
import numpy as np
from contextlib import ExitStack

import concourse.bass as bass
import concourse.tile as tile
from concourse import mybir, bacc
from concourse.bass_utils import run_bass_kernel_spmd
from concourse.masks import make_identity

DT = mybir.dt
AFT = mybir.ActivationFunctionType
ALU = mybir.AluOpType

N_CORES = 8
B, S, D, F, E = 4, 2048, 1024, 4096, 4
T = (B * S) // N_CORES          # 1024 tokens per core
P = 128
DC = D // P                     # 8 d-chunks
FC = F // P                     # 32 f-chunks
TC = T // P                     # 8 token chunks of 128
NTOK = 512                      # token half (max fp32 moving dim / psum bank)
NT = T // NTOK                  # 2 token halves
FG = 4                          # f-chunks per W1 psum group
MM_DT = DT.float32r             # TF32-rate matmuls for the FFN

_CACHE = {}

def _moe_kernel(tc, x, wr, br, w1, b1, w2, b2, out, gsc):
    nc = tc.nc
    with ExitStack() as ctx:
        singles = ctx.enter_context(tc.tile_pool(name="singles", bufs=1))
        ident = singles.tile([P, P], DT.float32)
        make_identity(nc, ident)

        wr_sb = singles.tile([P, DC, E], DT.float32)
        br_sb = singles.tile([E, 1], DT.float32)
        b2t_sb = singles.tile([P, E, DC], DT.float32)
        b1_sb = singles.tile([P, E, FC], DT.float32)
        ones_sb = singles.tile([1, P], DT.float32)
        nc.vector.memset(ones_sb, 1.0)
        L_row = singles.tile([E, T], DT.float32)
        g_row = singles.tile([E, T], DT.float32)
        # single-partition copy: PE matmul operands must start at partition
        # 0/32/64, so per-expert rows are staged on partition 0 for the
        # G-broadcast matmuls.
        g_row1 = singles.tile([1, E, T], DT.float32)

        # long-lived activations
        xt_pool = ctx.enter_context(tc.tile_pool(name="xt", bufs=1))
        xT = [[xt_pool.tile([P, NTOK], MM_DT, name=f"xT{d}_{n}") for n in range(NT)]
              for d in range(DC)]
        g_pool = ctx.enter_context(tc.tile_pool(name="gpool", bufs=1))
        G = [[g_pool.tile([P, NTOK], DT.float32, name=f"G{e}_{n}")
              for n in range(NT)] for e in range(E)]
        acc_pool = ctx.enter_context(tc.tile_pool(name="acc", bufs=1))
        accs = [acc_pool.tile([P, T], DT.float32, name=f"acc{d}") for d in range(DC)]

        # ---- phase 1: load x, transpose to xT (fp32r) + xTf (fp32, router) ----
        with ExitStack() as ctx2:
            xf_pool = ctx2.enter_context(tc.tile_pool(name="xf", bufs=1))
            xTf = [[xf_pool.tile([P, NTOK], DT.float32, name=f"xTf{d}_{n}")
                    for n in range(NT)] for d in range(DC)]
            xs_pool = ctx2.enter_context(tc.tile_pool(name="xs", bufs=3))
            tp_pool = ctx2.enter_context(tc.tile_pool(name="tp", bufs=4, space="PSUM"))
            for ti in range(TC):
                x_t = xs_pool.tile([P, D], DT.float32, name="x_t")
                nc.sync.dma_start(x_t[:], x[ti * P:(ti + 1) * P, :])
                nh = ti // (TC // NT)
                co = (ti % (TC // NT)) * P
                for d in range(DC):
                    tp = tp_pool.tile([P, P], DT.float32, name="tp")
                    nc.tensor.transpose(tp[:], x_t[:, d * P:(d + 1) * P], ident[:])
                    nc.scalar.copy(xT[d][nh][:, co:co + P], tp[:])
                    nc.vector.tensor_copy(xTf[d][nh][:, co:co + P], tp[:])

            # constants are loaded after the x tiles so the PE-blocking x
            # DMAs get served first; the 4-byte-granular b1 rearrange DMA in
            # particular is slow and is not needed until the first gelu.
            for cc in range(DC):
                nc.sync.dma_start(wr_sb[:, cc, :], wr[cc * P:(cc + 1) * P, :])
            nc.sync.dma_start(br_sb[:, :], br.unsqueeze(1))
            nc.sync.dma_start(b2t_sb[:], b2.rearrange("e (c p) -> p e c", p=P))
            nc.sync.dma_start(b1_sb[:], b1.rearrange("e (c p) -> p e c", p=P))

            # ---- phase 2: router logits in fp32 ----
            lg_pool = ctx2.enter_context(tc.tile_pool(name="lg", bufs=2, space="PSUM"))
            for n in range(NT):
                lp = lg_pool.tile([E, NTOK], DT.float32, name="lp")
                for d in range(DC):
                    nc.tensor.matmul(lp[:], wr_sb[:, d, :], xTf[d][n][:],
                                     start=(d == 0), stop=(d == DC - 1))
                nc.scalar.activation(L_row[:, n * NTOK:(n + 1) * NTOK], lp[:],
                                     AFT.Identity, bias=br_sb[:], scale=1.0)
        # xTf / x stream freed here

        # ---- phase 3: router math in token space ----
        rt = ctx.enter_context(tc.tile_pool(name="rt", bufs=1))
        with ExitStack() as ctx3:
            tpr = ctx3.enter_context(tc.tile_pool(name="tpr", bufs=2, space="PSUM"))
            Lt = rt.tile([P, TC, E], DT.float32)
            for ti in range(TC):
                tp = tpr.tile([P, E], DT.float32, name="tpr")
                nc.tensor.transpose(tp[:], L_row[:, ti * P:(ti + 1) * P],
                                    ident[0:E, 0:E])
                nc.scalar.copy(Lt[:, ti, :], tp[:])
            z = rt.tile([P, TC, E], DT.float32)
            nc.scalar.activation(z[:], Lt[:], AFT.Exp)
            # pairwise "a beats b" for a<b (ties -> lower index wins, as top_k)
            pairs = [(0, 1), (0, 2), (0, 3), (1, 2), (1, 3), (2, 3)]
            c = {}
            for (a, b_) in pairs:
                t = rt.tile([P, TC], DT.float32, name=f"c{a}{b_}")
                nc.vector.tensor_tensor(t[:], Lt[:, :, a], Lt[:, :, b_], ALU.is_ge)
                c[(a, b_)] = t
            s = [rt.tile([P, TC], DT.float32, name=f"s{e}") for e in range(E)]
            # s_e = number of wins of expert e; mask = s_e >= 2
            nc.vector.tensor_add(s[0][:], c[(0, 1)][:], c[(0, 2)][:])
            nc.vector.tensor_add(s[0][:], s[0][:], c[(0, 3)][:])
            nc.vector.tensor_add(s[1][:], c[(1, 2)][:], c[(1, 3)][:])
            nc.vector.tensor_sub(s[1][:], s[1][:], c[(0, 1)][:])
            nc.vector.tensor_scalar_add(s[1][:], s[1][:], 1.0)
            nc.vector.tensor_sub(s[2][:], c[(2, 3)][:], c[(0, 2)][:])
            nc.vector.tensor_sub(s[2][:], s[2][:], c[(1, 2)][:])
            nc.vector.tensor_scalar_add(s[2][:], s[2][:], 2.0)
            nc.vector.tensor_add(s[3][:], c[(0, 3)][:], c[(1, 3)][:])
            nc.vector.tensor_add(s[3][:], s[3][:], c[(2, 3)][:])
            nc.vector.tensor_scalar(s[3][:], s[3][:], -1.0, 3.0, ALU.mult, ALU.add)
            zm = rt.tile([P, TC, E], DT.float32)
            for e in range(E):
                nc.vector.tensor_single_scalar(zm[:, :, e], s[e][:], 1.5, ALU.is_ge)
            nc.vector.tensor_mul(zm[:], zm[:], z[:])
            den = rt.tile([P, TC], DT.float32)
            nc.vector.tensor_reduce(den[:], zm[:], axis=mybir.AxisListType.X,
                                    op=ALU.add)
            rec = rt.tile([P, TC], DT.float32)
            nc.vector.reciprocal(rec[:], den[:])
            gt = rt.tile([P, TC, E], DT.float32)
            for e in range(E):
                nc.vector.tensor_mul(gt[:, :, e], zm[:, :, e], rec[:])

        # ---- phase 5: expert FFN, dense, fp32r ----
        # F in halves (h for a half fits SBUF at full token width); W1/W2 each
        # read exactly once. PSUM is split 4+4 between the W1 and W2 phases so
        # consecutive phases pipeline; W2 weight DMAs issue from the scalar
        # engine to spread descriptor-generation load.
        ctx5 = ExitStack()
        hp = ctx5.enter_context(tc.tile_pool(name="hp", bufs=1))
        w1s = ctx5.enter_context(tc.tile_pool(name="w1s", bufs=8))
        w2s = ctx5.enter_context(tc.tile_pool(name="w2s", bufs=8))
        psw1 = ctx5.enter_context(tc.tile_pool(name="psw1", bufs=4, space="PSUM"))
        cmb = ctx5.enter_context(tc.tile_pool(name="cmb", bufs=4))
        psw2 = None
        FH = FC // 2                    # 16 f-chunks per half
        FG1 = 2                         # f-chunks per W1 psum group (2f x 2n)
        for e in range(E):
            for fh in range(2):
                f0 = fh * FH
                h_tiles = [hp.tile([P, T], MM_DT, name=f"h{f}") for f in range(FH)]
                for fg in range(FH // FG1):
                    hps = [[psw1.tile([P, NTOK], DT.float32, name="hps", tag="psa")
                            for _ in range(NT)] for _ in range(FG1)]
                    for d in range(DC):
                        w1_t = w1s.tile([P, FG1 * P], MM_DT, name="w1t")
                        fbase = (f0 + fg * FG1) * P
                        nc.sync.dma_start(
                            w1_t[:], w1[e, d * P:(d + 1) * P, fbase:fbase + FG1 * P])
                        for f in range(FG1):
                            for n in range(NT):
                                nc.tensor.matmul(
                                    hps[f][n][:], w1_t[:, f * P:(f + 1) * P],
                                    xT[d][n][:],
                                    start=(d == 0), stop=(d == DC - 1))
                    for f in range(FG1):
                        fi = fg * FG1 + f
                        for n in range(NT):
                            nc.scalar.activation(
                                h_tiles[fi][:, n * NTOK:(n + 1) * NTOK],
                                hps[f][n][:], AFT.Gelu,
                                bias=b1_sb[:, e, f0 + fi:f0 + fi + 1], scale=1.0)
                if e == 0 and fh == 0:
                    # G broadcast tiles + b2-combo acc seeds, off the startup
                    # critical path: PE does this while DMA prefetches W2.
                    with tc.tile_pool(name="gtemp", bufs=2, space="PSUM") as gtemp:
                        for ti in range(TC):
                            tp = gtemp.tile([E, P], DT.float32, name="tpg")
                            nc.tensor.transpose(tp[:], gt[:, ti, :], ident[:])
                            nc.scalar.copy(g_row[:, ti * P:(ti + 1) * P], tp[:])
                        nc.sync.dma_start(gsc[:, :], g_row[:, :])
                        for ge in range(E):
                            for n in range(NT):
                                bcast = bass.AP(
                                    tensor=gsc.tensor,
                                    offset=ge * T + n * NTOK,
                                    ap=[[0, P], [1, NTOK]])
                                nc.sync.dma_start(G[ge][n][:], bcast)
                        # b2-combo acc seeds on DVE: acc = sum_e b2[e,d] * G[e]
                        for d in range(DC):
                            for n in range(NT):
                                asl = accs[d][:, n * NTOK:(n + 1) * NTOK]
                                nc.vector.tensor_scalar_mul(
                                    asl, G[0][n][:], b2t_sb[:, 0, d:d + 1])
                                for ge in range(1, E):
                                    nc.vector.scalar_tensor_tensor(
                                        asl, G[ge][n][:], b2t_sb[:, ge, d:d + 1],
                                        asl, ALU.mult, ALU.add)
                if psw2 is None:
                    psw2 = ctx5.enter_context(
                        tc.tile_pool(name="psw2", bufs=4, space="PSUM"))
                # W2: acc += G[e] * (W2^T h), d-groups of (2d x 2n) psums
                for dg in range(DC // 2):
                    ops = [[psw2.tile([P, NTOK], DT.float32, name="ops", tag="psb")
                            for _ in range(NT)] for _ in range(2)]
                    for fk in range(FH):
                        w2_t = w2s.tile([P, 2 * P], MM_DT, name="w2t")
                        nc.gpsimd.dma_start(
                            w2_t[:], w2[e, (f0 + fk) * P:(f0 + fk + 1) * P,
                                        dg * 2 * P:(dg + 1) * 2 * P])
                        for dd in range(2):
                            for n in range(NT):
                                nc.tensor.matmul(
                                    ops[dd][n][:], w2_t[:, dd * P:(dd + 1) * P],
                                    h_tiles[fk][:, n * NTOK:(n + 1) * NTOK],
                                    start=(fk == 0), stop=(fk == FH - 1))
                    for dd in range(2):
                        d = dg * 2 + dd
                        for n in range(NT):
                            t = cmb.tile([P, NTOK], DT.float32, name="cmbt")
                            nc.vector.tensor_mul(t[:], ops[dd][n][:], G[e][n][:])
                            nc.vector.tensor_add(
                                accs[d][:, n * NTOK:(n + 1) * NTOK],
                                accs[d][:, n * NTOK:(n + 1) * NTOK], t[:])

        ctx5.close()

        # ---- phase 6: transpose acc back to [T, D] and store ----
        with tc.tile_pool(name="ot", bufs=3) as ot_pool, \
             tc.tile_pool(name="tpo", bufs=4, space="PSUM") as tpo:
            for ti in range(TC):
                o_t = ot_pool.tile([P, D], DT.float32, name="o_t")
                for d in range(DC):
                    tp = tpo.tile([P, P], DT.float32, name="tpo")
                    nc.tensor.transpose(tp[:], accs[d][:, ti * P:(ti + 1) * P],
                                        ident[:])
                    nc.scalar.copy(o_t[:, d * P:(d + 1) * P], tp[:])
                nc.sync.dma_start(out[ti * P:(ti + 1) * P, :], o_t[:])


def _build():
    nc = bacc.Bacc("TRN2", target_bir_lowering=False, debug=False,
                   num_devices=N_CORES)
    x = nc.dram_tensor("x", [T, D], DT.float32, kind="ExternalInput").ap()
    wr = nc.dram_tensor("wr", [D, E], DT.float32, kind="ExternalInput").ap()
    br_ = nc.dram_tensor("br", [E], DT.float32, kind="ExternalInput").ap()
    w1 = nc.dram_tensor("w1", [E, D, F], MM_DT, kind="ExternalInput").ap()
    b1 = nc.dram_tensor("b1", [E, F], DT.float32, kind="ExternalInput").ap()
    w2 = nc.dram_tensor("w2", [E, F, D], MM_DT, kind="ExternalInput").ap()
    b2 = nc.dram_tensor("b2", [E, D], DT.float32, kind="ExternalInput").ap()
    out = nc.dram_tensor("out", [T, D], DT.float32, kind="ExternalOutput").ap()
    gsc = nc.dram_tensor("g_scratch", [E, T], DT.float32).ap()
    with tile.TileContext(nc) as tc:
        _moe_kernel(tc, x, wr, br_, w1, b1, w2, b2, out, gsc)
    nc.finalize()
    return nc


def get_nc():
    if "nc" not in _CACHE:
        _CACHE["nc"] = _build()
    return _CACHE["nc"]


def kernel(x, Wr, br, W1, b1, W2, b2):
    x = np.ascontiguousarray(np.asarray(x, dtype=np.float32))
    Wr = np.ascontiguousarray(np.asarray(Wr, dtype=np.float32))
    br = np.ascontiguousarray(np.asarray(br, dtype=np.float32))
    W1 = np.ascontiguousarray(np.asarray(W1, dtype=np.float32))
    b1 = np.ascontiguousarray(np.asarray(b1, dtype=np.float32))
    W2 = np.ascontiguousarray(np.asarray(W2, dtype=np.float32))
    b2 = np.ascontiguousarray(np.asarray(b2, dtype=np.float32))

    nc = get_nc()
    xf = x.reshape(B * S, D)
    in_maps = []
    for cid in range(N_CORES):
        in_maps.append({
            "x": xf[cid * T:(cid + 1) * T],
            "wr": Wr, "br": br, "w1": W1, "b1": b1, "w2": W2, "b2": b2,
        })
    res = run_bass_kernel_spmd(nc, in_maps, core_ids=list(range(N_CORES)))
    out = np.concatenate([res.results[cid]["out"] for cid in range(N_CORES)],
                         axis=0)
    return out.reshape(B, S, D)



# revision 2
# speedup vs baseline: 1.7582x; 1.7582x over previous
"""MoE FFN (D=1024, F=4096, E=4, top-2) Trainium2 Bass kernel.

Strategy: expert-parallel dispatch. The router (a 8192x1024x4 matmul +
softmax + top-2) is computed on host in float64 -- it is 0.01% of the
model FLOPs and its only role is to pick the token->expert assignment
that defines the sharding.  Each expert is served by 2 of the 8 cores;
the host gathers each core's assigned tokens into a transposed
[D, C] activation block (capacity C, zero-padded), so the device kernel
is a dense single-expert FFN with tokens on the free dimension:

  h[F, C]  = gelu(W1^T @ xT + b1)        (fp32r matmuls, ACT gelu)
  oT[D, C] = g * (W2^T @ h + b2)         (fp32r matmuls, DVE combine)

The host then scatter-adds the two gated expert outputs per token.
Only the 2 selected experts per token are ever computed (2x fewer
matmul FLOPs than the dense reference), weights are read from HBM
exactly once, and there are no on-device transposes.

fp32r (TF32) runs the 128x128 PE at 1 cycle/row vs 4 for fp32.
"""
import numpy as np
from contextlib import ExitStack

import concourse.bass as bass
import concourse.tile as tile
from concourse import mybir, bacc
from concourse.bass_utils import run_bass_kernel_spmd

DT = mybir.dt
AFT = mybir.ActivationFunctionType
ALU = mybir.AluOpType

N_CORES = 8
B, S, D, F, E = 4, 2048, 1024, 4096, 4
TOKENS = B * S                  # 8192 tokens, 16384 (token, expert) jobs
TOP_K = 2
P = 128
DC = D // P                     # 8 d-chunks
FC = F // P                     # 32 f-chunks
NCC = 5                         # token chunks per core (each <= 512 psum cols)
C_DEFAULT = 2120                # per-core job capacity (seed-0 max load: 2101)
NPH = 8                         # F phases; FPH f-chunks of h live at a time
FPH = FC // NPH                 # 4
MM_DT = DT.float32r

_CACHE = {}


def _ffn(tc, xT, w1, b1, w2, b2, g, out, C):
    nc = tc.nc
    CC = C // NCC
    with ExitStack() as ctx:
        singles = ctx.enter_context(tc.tile_pool(name="singles", bufs=1))
        b1_sb = singles.tile([P, FC], DT.float32)
        b2_sb = singles.tile([P, DC], DT.float32)
        G = singles.tile([P, C], DT.float32)

        # resident activations: xT (input, fp32r) and acc (fp32 partials)
        xp = ctx.enter_context(tc.tile_pool(name="xp", bufs=1))
        xts = [[xp.tile([P, CC], MM_DT, name=f"x{d}_{c}") for c in range(NCC)]
               for d in range(DC)]
        accp = ctx.enter_context(tc.tile_pool(name="acc", bufs=1))
        acc = [accp.tile([P, C], DT.float32, name=f"acc{d}") for d in range(DC)]

        # xT first (PE-blocking), chunk-major so the first psum chain can
        # start after ~1.7MB instead of the full 8.5MB
        for c in range(NCC):
            for d in range(DC):
                nc.sync.dma_start(xts[d][c][:],
                                  xT[d * P:(d + 1) * P, c * CC:(c + 1) * CC])
        nc.sync.dma_start(b1_sb[:], b1.rearrange("(f p) -> p f", p=P))
        nc.sync.dma_start(b2_sb[:], b2.rearrange("(d p) -> p d", p=P))
        # gate row broadcast over all 128 partitions (stride-0 DMA)
        nc.gpsimd.dma_start(G[:], bass.AP(tensor=g.tensor, offset=0,
                                          ap=[[0, P], [1, C]]))

        hp = ctx.enter_context(tc.tile_pool(name="hp", bufs=1))
        h = [hp.tile([P, C], MM_DT, name=f"h{f}") for f in range(FPH)]
        w1p = ctx.enter_context(tc.tile_pool(name="w1p", bufs=2 * DC))
        w2p = ctx.enter_context(tc.tile_pool(name="w2p", bufs=2 * FPH))
        ps1 = ctx.enter_context(tc.tile_pool(name="ps1", bufs=4, space="PSUM"))
        ps2 = ctx.enter_context(tc.tile_pool(name="ps2", bufs=4, space="PSUM"))
        op = ctx.enter_context(tc.tile_pool(name="op", bufs=4))

        for ph in range(NPH):
            # ---- W1 slab: h[fi] = gelu(W1[:, slab]^T xT + b1) ----
            for fi in range(FPH):
                fg = ph * FPH + fi
                w1t = [w1p.tile([P, P], MM_DT, name="w1t") for _ in range(DC)]
                for d in range(DC):
                    nc.sync.dma_start(w1t[d][:],
                                      w1[d * P:(d + 1) * P, fg * P:(fg + 1) * P])
                for c in range(NCC):
                    pt = ps1.tile([P, CC], DT.float32, name="pt")
                    for d in range(DC):
                        nc.tensor.matmul(pt[:], w1t[d][:], xts[d][c][:],
                                         start=(d == 0), stop=(d == DC - 1))
                    nc.scalar.activation(h[fi][:, c * CC:(c + 1) * CC], pt[:],
                                         AFT.Gelu, bias=b1_sb[:, fg:fg + 1],
                                         scale=1.0)
            # ---- W2 slab: acc[d] += W2[slab, :]^T h ----
            for d in range(DC):
                w2t = [w2p.tile([P, P], MM_DT, name="w2t") for _ in range(FPH)]
                for fi in range(FPH):
                    fg = ph * FPH + fi
                    nc.gpsimd.dma_start(w2t[fi][:],
                                        w2[fg * P:(fg + 1) * P, d * P:(d + 1) * P])
                for c in range(NCC):
                    pt = ps2.tile([P, CC], DT.float32, name="pt2")
                    for fi in range(FPH):
                        nc.tensor.matmul(pt[:], w2t[fi][:],
                                         h[fi][:, c * CC:(c + 1) * CC],
                                         start=(fi == 0), stop=(fi == FPH - 1))
                    csl = slice(c * CC, (c + 1) * CC)
                    if ph == 0:
                        # seed acc with b2 while copying out of psum
                        nc.scalar.activation(acc[d][:, csl], pt[:], AFT.Identity,
                                             bias=b2_sb[:, d:d + 1], scale=1.0)
                    elif ph < NPH - 1:
                        nc.vector.tensor_add(acc[d][:, csl], acc[d][:, csl], pt[:])
                    else:
                        t = op.tile([P, CC], DT.float32, name="ot")
                        nc.vector.tensor_add(t[:], acc[d][:, csl], pt[:])
                        nc.vector.tensor_mul(t[:], t[:], G[:, csl])
                        nc.gpsimd.dma_start(out[d * P:(d + 1) * P, csl], t[:])


def _build(C):
    nc = bacc.Bacc("TRN2", target_bir_lowering=False, debug=False,
                   num_devices=N_CORES)
    xT = nc.dram_tensor("xt", [D, C], MM_DT, kind="ExternalInput").ap()
    w1 = nc.dram_tensor("w1", [D, F], MM_DT, kind="ExternalInput").ap()
    b1 = nc.dram_tensor("b1", [F], DT.float32, kind="ExternalInput").ap()
    w2 = nc.dram_tensor("w2", [F, D], MM_DT, kind="ExternalInput").ap()
    b2 = nc.dram_tensor("b2", [D], DT.float32, kind="ExternalInput").ap()
    g = nc.dram_tensor("g", [C], DT.float32, kind="ExternalInput").ap()
    out = nc.dram_tensor("out", [D, C], DT.float32, kind="ExternalOutput").ap()
    with tile.TileContext(nc) as tc:
        _ffn(tc, xT, w1, b1, w2, b2, g, out, C)
    nc.finalize()
    return nc


def get_nc(C=C_DEFAULT):
    if C not in _CACHE:
        _CACHE[C] = _build(C)
    return _CACHE[C]


def route(x, Wr, br):
    """Host router in float64: top-2 expert ids + renormalized gates.

    The rank2/rank3 prob gap is >=2.8e-5 on this data, so any router
    accurate to ~1e-6 (f64 trivially is) selects the same experts as the
    f32 reference; gate values agree to ~3e-6.
    """
    xf = x.reshape(TOKENS, D).astype(np.float64)
    logits = xf @ Wr.astype(np.float64) + br.astype(np.float64)
    m = logits.max(axis=-1, keepdims=True)
    ez = np.exp(logits - m)
    probs = ez / ez.sum(axis=-1, keepdims=True)
    order = np.argsort(-probs, axis=-1, kind="stable")
    top2 = order[:, :TOP_K]
    p2 = np.take_along_axis(probs, top2, axis=1)
    gates = (p2 / p2.sum(axis=-1, keepdims=True)).astype(np.float32)
    return top2, gates


def dispatch(x, Wr, br):
    """Token->core assignment: expert e is served by cores 2e and 2e+1."""
    top2, gates = route(x, Wr, br)
    toks, gvals = [], []
    for e in range(E):
        hit = top2 == e                        # (TOKENS, 2)
        te = np.nonzero(hit.any(axis=1))[0]
        ge = (gates * hit).sum(axis=1)[te].astype(np.float32)
        n = len(te)
        half = (n + 1) // 2
        toks.extend([te[:half], te[half:]])
        gvals.extend([ge[:half], ge[half:]])
    return toks, gvals


def make_in_maps(inputs, C=C_DEFAULT):
    x = np.ascontiguousarray(np.asarray(inputs["x"], dtype=np.float32))
    Wr = np.asarray(inputs["Wr"], dtype=np.float32)
    br = np.asarray(inputs["br"], dtype=np.float32)
    W1 = np.ascontiguousarray(np.asarray(inputs["W1"], dtype=np.float32))
    b1 = np.ascontiguousarray(np.asarray(inputs["b1"], dtype=np.float32))
    W2 = np.ascontiguousarray(np.asarray(inputs["W2"], dtype=np.float32))
    b2 = np.ascontiguousarray(np.asarray(inputs["b2"], dtype=np.float32))

    toks, gvals = dispatch(x, Wr, br)
    needed = max(len(t) for t in toks)
    if needed > C:
        C = ((needed + NCC * 8 - 1) // (NCC * 8)) * NCC * 8  # NCC-divisible

    xTfull = np.ascontiguousarray(x.reshape(TOKENS, D).T)
    in_maps = []
    for cid in range(N_CORES):
        e = cid // 2
        cnt = len(toks[cid])
        xt_c = np.zeros((D, C), dtype=np.float32)
        xt_c[:, :cnt] = xTfull[:, toks[cid]]
        g_c = np.zeros((C,), dtype=np.float32)
        g_c[:cnt] = gvals[cid]
        in_maps.append({"xt": xt_c, "w1": W1[e], "b1": b1[e],
                        "w2": W2[e], "b2": b2[e], "g": g_c})
    return in_maps, toks, C


def kernel(x, Wr, br, W1, b1, W2, b2):
    inputs = {"x": x, "Wr": Wr, "br": br, "W1": W1, "b1": b1,
              "W2": W2, "b2": b2}
    in_maps, toks, C = make_in_maps(inputs)
    nc = get_nc(C)
    res = run_bass_kernel_spmd(nc, in_maps, core_ids=list(range(N_CORES)))
    outT = np.zeros((D, TOKENS), dtype=np.float32)
    for cid in range(N_CORES):
        cnt = len(toks[cid])
        outT[:, toks[cid]] += res.results[cid]["out"][:, :cnt]
    return np.ascontiguousarray(outT.T).reshape(B, S, D)


# revision 8
# speedup vs baseline: 2.1860x; 1.2433x over previous
"""MoE FFN (D=1024, F=4096, E=4, top-2) Trainium2 Bass kernel.

Strategy: expert-parallel dispatch. The router (a 8192x1024x4 matmul +
softmax + top-2) is computed on host in float64 -- it is 0.01% of the
model FLOPs and its only role is to pick the token->expert assignment
that defines the sharding.  Each expert is served by 2 of the 8 cores;
the host gathers each core's assigned tokens into a transposed
[D, C] activation block (capacity C, zero-padded), so the device kernel
is a dense single-expert FFN with tokens on the free dimension:

  h[F, C]  = gelu(W1^T @ xT + b1)        (bf16 matmuls, ACT gelu)
  oT[D, C] = g * (W2^T @ h + b2)         (bf16 matmuls, fp32 psum/acc)

The host then scatter-adds the two gated expert outputs per token.
Only the 2 selected experts per token are ever computed (2x fewer
matmul FLOPs than the dense reference), weights are read from HBM
exactly once, and there are no on-device transposes.

bf16 runs the PE at 1 cycle/row (same as fp32r) but halves LDWEIGHTS
time (stationary reload is the per-matmul overhead) and all weight DMA.
PSUM accumulation stays fp32; only matmul operand storage is bf16.

DMA queues are specialized so nothing blocks the critical path:
gpsimd = biases/gates + weight slabs, sync = xT + output, and the
scalar engine issues no DMA (gelu drains psum on the critical path).
"""
import numpy as np
import ml_dtypes
from contextlib import ExitStack

import concourse.bass as bass
import concourse.tile as tile
from concourse import mybir, bacc
from concourse.bass_utils import run_bass_kernel_spmd

DT = mybir.dt
AFT = mybir.ActivationFunctionType
ALU = mybir.AluOpType

N_CORES = 8
B, S, D, F, E = 4, 2048, 1024, 4096, 4
TOKENS = B * S                  # 8192 tokens, 16384 (token, expert) jobs
TOP_K = 2
P = 128
DC = D // P                     # 8 d-chunks
FC = F // P                     # 32 f-chunks
NCC = 5                         # token chunks per core (each <= 512 psum cols)
C_DEFAULT = 2120                # per-core job capacity (seed-0 max load: 2101)
NPH = 4                         # F phases; FPH f-chunks of h live at a time
FPH = FC // NPH                 # 8
MM_DT = DT.bfloat16
NP_MM = ml_dtypes.bfloat16

_CACHE = {}


def _ffn(tc, xT, w1, b1, w2, b2, g, out, C):
    nc = tc.nc
    CC = C // NCC
    FW = FPH * P                # 1024 f columns per W1 phase slab
    with ExitStack() as ctx:
        singles = ctx.enter_context(tc.tile_pool(name="singles", bufs=1))
        b1_sb = singles.tile([P, FC], DT.float32)
        b2_sb = singles.tile([P, DC], DT.float32)
        G = singles.tile([P, C], DT.float32)

        # resident activations: xT (input, bf16) and acc (fp32 partials)
        xp = ctx.enter_context(tc.tile_pool(name="xp", bufs=1))
        xts = [[xp.tile([P, CC], MM_DT, name=f"x{d}_{c}") for c in range(NCC)]
               for d in range(DC)]
        accp = ctx.enter_context(tc.tile_pool(name="acc", bufs=1))
        acc = [accp.tile([P, C], DT.float32, name=f"acc{d}") for d in range(DC)]

        # small constants first (gpsimd queue), then xT chunk-major on the
        # sync/vector queues so the first psum chain can start after ~1MB
        nc.gpsimd.dma_start(b1_sb[:], b1.rearrange("(f p) -> p f", p=P))
        nc.gpsimd.dma_start(b2_sb[:], b2.rearrange("(d p) -> p d", p=P))
        nc.gpsimd.dma_start(G[:], bass.AP(tensor=g.tensor, offset=0,
                                          ap=[[0, P], [1, C]]))
        for c in range(NCC):
            for d in range(DC):
                nc.sync.dma_start(xts[d][c][:],
                                  xT[d * P:(d + 1) * P, c * CC:(c + 1) * CC])

        hp = ctx.enter_context(tc.tile_pool(name="hp", bufs=1))
        h = [hp.tile([P, C], MM_DT, name=f"h{f}") for f in range(FPH)]
        # batched weight slabs: W1 [128d x 1024f] per (phase, d);
        # W2 [128f x 1024d(=D)] per f-chunk.  One contiguous DMA each.
        w1p = ctx.enter_context(tc.tile_pool(name="w1p", bufs=10))
        w2p = ctx.enter_context(tc.tile_pool(name="w2p", bufs=10))
        ps1 = ctx.enter_context(tc.tile_pool(name="ps1", bufs=4, space="PSUM"))
        ps2 = ctx.enter_context(tc.tile_pool(name="ps2", bufs=4, space="PSUM"))
        op = ctx.enter_context(tc.tile_pool(name="op", bufs=4))

        for ph in range(NPH):
            f0 = ph * FPH
            # ---- W1 slab: h[fi] = gelu(W1[:, slab]^T xT + b1) ----
            w1t = [w1p.tile([P, FW], MM_DT, name="w1t") for _ in range(DC)]
            for d in range(DC):
                nc.gpsimd.dma_start(w1t[d][:],
                                    w1[d * P:(d + 1) * P, f0 * P:f0 * P + FW])
            w2t = [w2p.tile([P, D], MM_DT, name="w2t") for _ in range(FPH)]
            for fi in range(FPH):
                fg = f0 + fi
                nc.gpsimd.dma_start(w2t[fi][:], w2[fg * P:(fg + 1) * P, :])
            for c in range(NCC):
                for fi in range(FPH):
                    pt = ps1.tile([P, CC], DT.float32, name="pt")
                    for d in range(DC):
                        nc.tensor.matmul(pt[:],
                                         w1t[d][:, fi * P:(fi + 1) * P],
                                         xts[d][c][:],
                                         start=(d == 0), stop=(d == DC - 1))
                    nc.scalar.activation(h[fi][:, c * CC:(c + 1) * CC], pt[:],
                                         AFT.Gelu, bias=b1_sb[:, f0 + fi:f0 + fi + 1],
                                         scale=1.0)
            # ---- W2 slab: acc[d] += W2[slab, :]^T h ----
            for d in range(DC):
                for c in range(NCC):
                    pt = ps2.tile([P, CC], DT.float32, name="pt2")
                    for fi in range(FPH):
                        nc.tensor.matmul(pt[:],
                                         w2t[fi][:, d * P:(d + 1) * P],
                                         h[fi][:, c * CC:(c + 1) * CC],
                                         start=(fi == 0), stop=(fi == FPH - 1))
                    csl = slice(c * CC, (c + 1) * CC)
                    if ph == 0:
                        # seed acc with b2 while copying out of psum
                        nc.scalar.activation(acc[d][:, csl], pt[:], AFT.Identity,
                                             bias=b2_sb[:, d:d + 1], scale=1.0)
                    elif ph < NPH - 1:
                        nc.vector.tensor_add(acc[d][:, csl], acc[d][:, csl], pt[:])
                    else:
                        t = op.tile([P, CC], DT.float32, name="ot")
                        nc.vector.tensor_add(t[:], acc[d][:, csl], pt[:])
                        nc.vector.tensor_mul(t[:], t[:], G[:, csl])
                        nc.sync.dma_start(out[d * P:(d + 1) * P, csl], t[:])


def _build(C):
    nc = bacc.Bacc("TRN2", target_bir_lowering=False, debug=False,
                   num_devices=N_CORES)
    xT = nc.dram_tensor("xt", [D, C], MM_DT, kind="ExternalInput").ap()
    w1 = nc.dram_tensor("w1", [D, F], MM_DT, kind="ExternalInput").ap()
    b1 = nc.dram_tensor("b1", [F], DT.float32, kind="ExternalInput").ap()
    w2 = nc.dram_tensor("w2", [F, D], MM_DT, kind="ExternalInput").ap()
    b2 = nc.dram_tensor("b2", [D], DT.float32, kind="ExternalInput").ap()
    g = nc.dram_tensor("g", [C], DT.float32, kind="ExternalInput").ap()
    out = nc.dram_tensor("out", [D, C], DT.float32, kind="ExternalOutput").ap()
    with tile.TileContext(nc) as tc:
        _ffn(tc, xT, w1, b1, w2, b2, g, out, C)
    nc.finalize()
    return nc


def get_nc(C=C_DEFAULT):
    if C not in _CACHE:
        _CACHE[C] = _build(C)
    return _CACHE[C]


def route(x, Wr, br):
    """Host router in float64: top-2 expert ids + renormalized gates.

    The rank2/rank3 prob gap is >=2.8e-5 on this data, so any router
    accurate to ~1e-6 (f64 trivially is) selects the same experts as the
    f32 reference; gate values agree to ~3e-6.
    """
    xf = x.reshape(TOKENS, D).astype(np.float64)
    logits = xf @ Wr.astype(np.float64) + br.astype(np.float64)
    m = logits.max(axis=-1, keepdims=True)
    ez = np.exp(logits - m)
    probs = ez / ez.sum(axis=-1, keepdims=True)
    order = np.argsort(-probs, axis=-1, kind="stable")
    top2 = order[:, :TOP_K]
    p2 = np.take_along_axis(probs, top2, axis=1)
    gates = (p2 / p2.sum(axis=-1, keepdims=True)).astype(np.float32)
    return top2, gates


def dispatch(x, Wr, br):
    """Token->core assignment: expert e is served by cores 2e and 2e+1."""
    top2, gates = route(x, Wr, br)
    toks, gvals = [], []
    for e in range(E):
        hit = top2 == e                        # (TOKENS, 2)
        te = np.nonzero(hit.any(axis=1))[0]
        ge = (gates * hit).sum(axis=1)[te].astype(np.float32)
        n = len(te)
        half = (n + 1) // 2
        toks.extend([te[:half], te[half:]])
        gvals.extend([ge[:half], ge[half:]])
    return toks, gvals


def make_in_maps(inputs, C=C_DEFAULT):
    x = np.ascontiguousarray(np.asarray(inputs["x"], dtype=np.float32))
    Wr = np.asarray(inputs["Wr"], dtype=np.float32)
    br = np.asarray(inputs["br"], dtype=np.float32)
    W1 = np.asarray(inputs["W1"], dtype=np.float32)
    b1 = np.ascontiguousarray(np.asarray(inputs["b1"], dtype=np.float32))
    W2 = np.asarray(inputs["W2"], dtype=np.float32)
    b2 = np.ascontiguousarray(np.asarray(inputs["b2"], dtype=np.float32))

    toks, gvals = dispatch(x, Wr, br)
    needed = max(len(t) for t in toks)
    if needed > C:
        C = ((needed + NCC * 8 - 1) // (NCC * 8)) * NCC * 8  # NCC-divisible

    xTfull = x.reshape(TOKENS, D).T.astype(NP_MM)  # [D, TOKENS] contiguous
    w1b = [np.ascontiguousarray(W1[e]).astype(NP_MM) for e in range(E)]
    w2b = [np.ascontiguousarray(W2[e]).astype(NP_MM) for e in range(E)]
    in_maps = []
    for cid in range(N_CORES):
        e = cid // 2
        cnt = len(toks[cid])
        xt_c = np.zeros((D, C), dtype=NP_MM)
        xt_c[:, :cnt] = xTfull[:, toks[cid]]
        g_c = np.zeros((C,), dtype=np.float32)
        g_c[:cnt] = gvals[cid]
        in_maps.append({"xt": xt_c, "w1": w1b[e], "b1": b1[e],
                        "w2": w2b[e], "b2": b2[e], "g": g_c})
    return in_maps, toks, C


def kernel(x, Wr, br, W1, b1, W2, b2):
    inputs = {"x": x, "Wr": Wr, "br": br, "W1": W1, "b1": b1,
              "W2": W2, "b2": b2}
    in_maps, toks, C = make_in_maps(inputs)
    nc = get_nc(C)
    res = run_bass_kernel_spmd(nc, in_maps, core_ids=list(range(N_CORES)))
    outT = np.zeros((D, TOKENS), dtype=np.float32)
    for cid in range(N_CORES):
        cnt = len(toks[cid])
        outT[:, toks[cid]] += res.results[cid]["out"][:, :cnt]
    return np.ascontiguousarray(outT.T).reshape(B, S, D)
